# revision 9
# baseline (speedup 1.0000x reference)
"""Trainium2 Bass kernel for nn_MultiHeadAttention (B=8, S=1024, DM=1024, H=16, D=64).

Sharding: data-parallel over batch - one batch element per NeuronCore (8 cores).

v2 design (vs v1): single PE-saturating pipeline.
  - Host pre-transposes Q/K/V to [DM, S] bf16 (kills on-device XBAR transposes
    and the ~30us startup bubble).
  - Key-pad masking moved entirely into host-zeroed VT columns + a pad-aware
    ones-column (vones) that feeds the softmax denominator: no kbias, no -1e9
    adds for padding. Causal masking = compile-time tile skip + post-exp
    multiply of the 128x128 boundary block by a 0/1 triangle.
  - Phase order: warmup-spam -> V-proj -> per-head-pair pipeline where pair
    c's scores/exp/attV interleave with pair c+1's Q/K projections, keeping
    PE busy while ACT does exp. Output projection computed transposed
    (outT[n,s]) so the bias add is a per-partition ACT op; host transposes
    back.
  - One PSUM pool for the whole kernel: prj 2x1 bank, scores 1x2 banks,
    xh 2x2 banks = 8 banks exactly.
pad==0 batches (reference quirk: uniform attention over ALL keys) are
computed on host and overwrite the device result for that batch.
"""

import os
import sys
import types
import ctypes
import contextlib
from collections import deque
import numpy as np
import ml_dtypes

import concourse.bass as bass
import concourse.mybir as mybir
from concourse import bacc
from concourse.tile import TileContext
from concourse.bass_utils import run_bass_kernel_spmd

FP32 = mybir.dt.float32
BF16 = mybir.dt.bfloat16
AF = mybir.ActivationFunctionType

B, S, DM, H, D, P = 8, 1024, 1024, 16, 64, 128
NCH = DM // P  # 8 chunks of 128
E = 65  # per-head attV stationary width: 64 value dims + 1 ones column
SCALE = 1.0 / 32.0  # 1/sqrt(S)
LAG = 4  # attV half-ops lag behind scores emission (2 whole cts)

_cache = {}


def _install_profile_shim():
    """Provide antenv.axon_hooks (NTFF profiling hook) when the image lacks it."""
    try:
        from antenv import axon_hooks  # noqa: F401
        return
    except ImportError:
        pass
    so_path = "/opt/axon/libaxon_pjrt.so"
    mod = types.ModuleType("antenv.axon_hooks")
    _state = {"hook": None}

    def set_axon_ntff_profile_hook(h):
        _state["hook"] = h

    def get_axon_ntff_profile_hook():
        return _state["hook"]

    mod.set_axon_ntff_profile_hook = set_axon_ntff_profile_hook
    mod.get_axon_ntff_profile_hook = get_axon_ntff_profile_hook
    sys.modules["antenv.axon_hooks"] = mod

    if not os.path.exists(so_path):
        return
    try:
        lib = ctypes.CDLL(so_path)
    except OSError:
        return
    if not hasattr(lib, "axon_start_nrt_profile"):
        return
    lib.axon_start_nrt_profile.argtypes = [
        ctypes.POINTER(ctypes.c_int64),
        ctypes.c_size_t,
    ]
    lib.axon_start_nrt_profile.restype = ctypes.c_int64
    lib.axon_stop_nrt_profile.argtypes = [ctypes.c_char_p]
    lib.axon_stop_nrt_profile.restype = ctypes.c_int64

    @contextlib.contextmanager
    def _hook(output_dir, device_ids):
        import jax

        jax.devices()
        if device_ids:
            ids = (ctypes.c_int64 * len(device_ids))(*device_ids)
            rc = lib.axon_start_nrt_profile(ids, len(device_ids))
        else:
            rc = lib.axon_start_nrt_profile(None, 0)
        if rc != 0:
            raise RuntimeError(f"axon_start_nrt_profile rc={rc}")
        try:
            yield
        finally:
            n = lib.axon_stop_nrt_profile(str(output_dir).encode())
            print(f"profile: {n} file(s) written to {output_dir}", file=sys.stderr)

    set_axon_ntff_profile_hook(_hook)


def _build():
    nc = bacc.Bacc()
    mul_eng_name = os.environ.get("KMUL_ENG", "vector")
    tri_eng_name = os.environ.get("KTRI_ENG", "vector")

    qtd = nc.dram_tensor("qt", [DM, S], BF16, kind="ExternalInput")
    ktd = nc.dram_tensor("kt", [DM, S], BF16, kind="ExternalInput")
    vtd = nc.dram_tensor("vt", [DM, S], BF16, kind="ExternalInput")
    wqd = nc.dram_tensor("wq", [DM, DM], BF16, kind="ExternalInput")
    wkd = nc.dram_tensor("wk", [DM, DM], BF16, kind="ExternalInput")
    wvd = nc.dram_tensor("wv", [DM, DM], BF16, kind="ExternalInput")
    wod = nc.dram_tensor("wo", [DM, DM], BF16, kind="ExternalInput")
    bqd = nc.dram_tensor("bqf", [DM], FP32, kind="ExternalInput")
    bkd = nc.dram_tensor("bkf", [DM], FP32, kind="ExternalInput")
    bopd = nc.dram_tensor("bop", [DM], FP32, kind="ExternalInput")
    vonesd = nc.dram_tensor("vones", [S, H], BF16, kind="ExternalInput")
    trid = nc.dram_tensor("tri2", [P, 2, P], BF16, kind="ExternalInput")
    outd = nc.dram_tensor("out", [DM, S], FP32, kind="ExternalOutput")

    with TileContext(nc) as tc:
        from contextlib import ExitStack

        with ExitStack() as ctx:
            mul_eng = nc.gpsimd if mul_eng_name == "gpsimd" else nc.vector
            tri_eng = nc.gpsimd if tri_eng_name == "gpsimd" else nc.vector

            ps = ctx.enter_context(tc.tile_pool(name="ps", bufs=1, space="PSUM"))
            const = ctx.enter_context(tc.tile_pool(name="const", bufs=1))
            tri_sb = const.tile([P, 2, P], BF16)
            nc.sync.dma_start(tri_sb, trid[:, :, :])
            bq_sb = const.tile([P, NCH], FP32)
            nc.sync.dma_start(bq_sb, bqd[:].rearrange("(c p) -> p c", p=P))
            bk_sb = const.tile([P, NCH], FP32)
            nc.sync.dma_start(bk_sb, bkd[:].rearrange("(c p) -> p c", p=P))
            bop_sb = const.tile([P, NCH], FP32)
            nc.sync.dma_start(bop_sb, bopd[:].rearrange("(c p) -> p c", p=P))

            # PE warmup: ~28 back-to-back tiny matmuls flip the HAM clock gate
            # to 8/8 while the first input DMAs land.
            wu = const.tile([P, P], BF16)
            nc.vector.memset(wu, 0.0)
            warm = ps.tile([P, 512], FP32, tag="prj", bufs=2)
            for _ in range(56):
                nc.tensor.matmul(warm[:, 0:P], lhsT=wu, rhs=wu, start=True, stop=True)

            # persistent SBUF
            vhp = ctx.enter_context(tc.tile_pool(name="vhp", bufs=1))
            Vh = [vhp.tile([P, H * E], BF16, name=f"vh{t}") for t in range(NCH)]
            xtp = ctx.enter_context(tc.tile_pool(name="xtp", bufs=1))
            XT = [xtp.tile([P, S], BF16, name=f"xt{c}") for c in range(NCH)]
            wkq = ctx.enter_context(tc.tile_pool(name="wkq", bufs=1))
            wq_sb = [wkq.tile([P, DM], BF16, name=f"wq{k}") for k in range(NCH)]
            wk_sb = [wkq.tile([P, DM], BF16, name=f"wk{k}") for k in range(NCH)]
            srcp = ctx.enter_context(tc.tile_pool(name="srcp", bufs=1))
            qt_sb = [srcp.tile([P, S], BF16, name=f"qts{k}") for k in range(NCH)]
            kt_sb = [srcp.tile([P, S], BF16, name=f"kts{k}") for k in range(NCH)]
            wop = ctx.enter_context(tc.tile_pool(name="wop", bufs=1))
            wo_sb = [wop.tile([P, DM], BF16, name=f"wo{k}") for k in range(NCH)]
            # bf16 partial sums for the two-pass output projection (pass 1 =
            # XT[0..6] contributions, computed during pair 7; pass 2 adds the
            # late XT[7] rank-128 update)
            dpp = ctx.enter_context(tc.tile_pool(name="dpp", bufs=1))
            dpart = {
                (n, half): dpp.tile([P, 512], BF16, name=f"dp{n}{half}")
                for n in range(NCH) for half in range(2)
            }

            # ---------------- V phase ----------------
            with ExitStack() as vctx:
                vpool = vctx.enter_context(tc.tile_pool(name="vpool", bufs=1))
                vt_sb = [vpool.tile([P, S], BF16, name=f"vts{k}") for k in range(NCH)]
                wv_sb = [vpool.tile([P, DM], BF16, name=f"wv{k}") for k in range(NCH)]
                for k in range(NCH):
                    nc.sync.dma_start(vt_sb[k], vtd[k * P:(k + 1) * P, :])
                    nc.sync.dma_start(wv_sb[k], wvd[k * P:(k + 1) * P, :])
                # stage the rest of the inputs behind V's DMAs
                for k in range(NCH):
                    nc.sync.dma_start(qt_sb[k], qtd[k * P:(k + 1) * P, :])
                for k in range(NCH):
                    nc.sync.dma_start(wq_sb[k], wqd[k * P:(k + 1) * P, :])
                for k in range(NCH):
                    nc.sync.dma_start(kt_sb[k], ktd[k * P:(k + 1) * P, :])
                for k in range(NCH):
                    nc.sync.dma_start(wk_sb[k], wkd[k * P:(k + 1) * P, :])

                for tt in range(NCH):
                    vh_view = Vh[tt].rearrange("p (h e) -> p h e", e=E)
                    for half in range(2):
                        prj = ps.tile([P, 512], FP32, tag="prj", bufs=2)
                        for kk in range(NCH):
                            nc.tensor.matmul(
                                prj,
                                lhsT=vt_sb[kk][:, tt * P:(tt + 1) * P],
                                rhs=wv_sb[kk][:, half * 512:(half + 1) * 512],
                                start=(kk == 0),
                                stop=(kk == NCH - 1),
                            )
                        nc.scalar.copy(
                            vh_view[:, half * 8:(half + 1) * 8, 0:64],
                            prj.rearrange("p (h e) -> p h e", e=64),
                        )
                    nc.sync.dma_start(
                        vh_view[:, :, 64:65],
                        vonesd[tt * P:(tt + 1) * P, :, None],
                    )

            # ---------------- pairs: attention ⊗ Q/K projections ----------------
            with ExitStack() as actx:
                qkh = actx.enter_context(tc.tile_pool(name="qkh", bufs=2))
                eps = actx.enter_context(tc.tile_pool(name="eps", bufs=10))
                epi = actx.enter_context(tc.tile_pool(name="epi", bufs=1))

                for k in range(NCH):
                    nc.sync.dma_start(wo_sb[k], wod[k * P:(k + 1) * P, :])

                qh_t, kh_t = {}, {}

                def proj_steps(c):
                    """Emission steps (closures) for pair c's Q/K projections."""
                    steps = []
                    for kind in ("q", "k"):
                        w_sb = wq_sb if kind == "q" else wk_sb
                        s_sb = qt_sb if kind == "q" else kt_sb
                        bias = bq_sb if kind == "q" else bk_sb
                        dst = qkh.tile([P, S], BF16, tag=f"{kind}ht", name=f"{kind}ht{c}")
                        (qh_t if kind == "q" else kh_t)[c] = dst
                        for half in range(2):
                            prj = ps.tile([P, 512], FP32, tag="prj", bufs=2,
                                          name=f"prj{kind}{c}{half}")

                            def mk_mm(kk0, prj=prj, w_sb=w_sb, s_sb=s_sb, c=c, half=half):
                                def f():
                                    for kk in (kk0, kk0 + 1):
                                        nc.tensor.matmul(
                                            prj,
                                            lhsT=w_sb[kk][:, c * P:(c + 1) * P],
                                            rhs=s_sb[kk][:, half * 512:(half + 1) * 512],
                                            start=(kk == 0),
                                            stop=(kk == NCH - 1),
                                        )
                                return f

                            for kk0 in range(0, NCH, 2):
                                steps.append(mk_mm(kk0))

                            def mk_cp(prj=prj, dst=dst, bias=bias, c=c, half=half):
                                def f():
                                    nc.vector.tensor_scalar_add(
                                        dst[:, half * 512:(half + 1) * 512],
                                        prj, bias[:, c:c + 1],
                                    )
                                return f

                            steps.append(mk_cp())
                    return steps

                def d1_steps():
                    """Pass-1 output-projection steps: pair 7's PE filler."""
                    steps = []
                    for n in range(NCH):
                        for half in range(2):
                            op = ps.tile([P, 512], FP32, tag="prj", bufs=2,
                                         name=f"op{n}{half}")

                            def mk(n=n, half=half, op=op):
                                def f():
                                    # kk 0..5 only: XT[6]/XT[7] are written by
                                    # ops emitted after these (pair 6/7
                                    # finish chains) — they join in pass 2
                                    for kk in range(6):
                                        nc.tensor.matmul(
                                            op,
                                            lhsT=wo_sb[kk][:, n * P:(n + 1) * P],
                                            rhs=XT[kk][:, half * 512:(half + 1) * 512],
                                            start=(kk == 0),
                                            stop=(kk == 5),
                                        )
                                    nc.scalar.activation(
                                        dpart[(n, half)], op, AF.Identity,
                                        bias=bop_sb[:, n:n + 1],
                                    )
                                return f

                            steps.append(mk())
                    return steps

                # prime pair 0's projections
                for st in proj_steps(0):
                    st()

                # Cross-pair software pipeline: `pend` carries attV half-ops
                # and each pair's epilogue as emission items, so the attV
                # stream of pair c drains during pair c+1's scores — no
                # pipeline bubble at pair boundaries. `deferred` carries the
                # denominator chain (broadcast/recip/mul), run one pair later
                # when its dr DMAs have long landed.
                pend = deque()
                deferred = deque()

                def emit_epilogue(c, xh):
                    # evacuate this pair's xh PSUM + extract denominator rows
                    xcp, drs = [], []
                    for hh in range(2):
                        x = epi.tile([E, S], FP32, tag=f"xcp{hh}", name=f"xcp{c}_{hh}")
                        if hh == 0:
                            nc.scalar.copy(x, xh[hh][0:E, :])
                        else:
                            nc.vector.tensor_copy(x, xh[hh][0:E, :])
                        xcp.append(x)
                    drp = epi.tile([P, 16], FP32, tag="drp", name=f"drp{c}")
                    for hh in range(2):
                        # [1,1024] row -> [128,8] packed (s = p*8 + j): the
                        # reciprocal then runs at 8 elems/lane, not 1024
                        nc.sync.dma_start(
                            drp[:, hh * 8:(hh + 1) * 8], xcp[hh][64:65, :]
                        )
                        drs.append(drp[:, hh * 8:(hh + 1) * 8])

                    # three age-staggered stages: by the time each stage is
                    # emitted, its inputs have landed, so no engine FIFO ever
                    # holds a long wait (which would block the attention
                    # pipeline's tri/mul ops queued behind it)
                    def stage3a(c, xcp, rbs, eng):
                        def f():
                            eng.tensor_mul(XT[c][0:64, :], xcp[0][0:64, :], rbs[0])
                        return f

                    def stage3b(c, xcp, rbs, eng):
                        def f():
                            stg = epi.tile([64, S], BF16, tag="stg", name=f"stg{c}")
                            eng.tensor_mul(stg, xcp[1][0:64, :], rbs[1])
                            nc.sync.dma_start(XT[c][64:128, :], stg)
                        return f

                    def stage2(c, xcp, rcp):
                        def f():
                            rbs = []
                            for hh in range(2):
                                r1 = epi.tile([1, S], FP32, tag=f"r1{hh}", name=f"r1{c}_{hh}")
                                nc.sync.dma_start(r1, rcp[:, hh * 8:(hh + 1) * 8])
                                rb = epi.tile([64, S], FP32, tag=f"rb{hh}", name=f"rb{c}_{hh}")
                                nc.gpsimd.partition_broadcast(rb, r1)
                                rbs.append(rb)
                            # last pair: DVE muls (shorter serial chain, and
                            # nothing queues behind them); others: gpsimd so
                            # the DVE FIFO never blocks the attention stream
                            eng = nc.vector if c == NCH - 1 else mul_eng
                            deferred.append([2, stage3a(c, xcp, rbs, eng)])
                            deferred.append([3, stage3b(c, xcp, rbs, eng)])
                        return f

                    def stage1(c=c, xcp=xcp, drp=drp):
                        rcp = epi.tile([P, 16], FP32, tag="rcp", name=f"rcp{c}")
                        nc.vector.reciprocal_approx_fast(rcp, drp)
                        deferred.append([2, stage2(c, xcp, rcp)])

                    deferred.append([2, stage1])

                d1_rest = []
                for c in range(NCH):
                    if c + 1 < NCH:
                        filler = deque(proj_steps(c + 1))
                    else:
                        # no filler for pair 7: pass 1 runs after the drain,
                        # where every prior pair's XT writes are emitted, and
                        # covers the last pair's normalize-chain latency
                        filler = deque()
                        d1_rest = d1_steps()
                    QhT, KhT = qh_t.pop(c), kh_t.pop(c)
                    xh = [
                        ps.tile([E, S], FP32, tag=f"xh{hh}", bufs=1, name=f"xh{c}_{hh}")
                        for hh in range(2)
                    ]
                    ex_store = {}

                    def emit_attv_half(ct, hh, c=c, xh=xh, ex_store=ex_store):
                        h = 2 * c + hh
                        for ext, sa, sb_ in ex_store[ct]:
                            if sa < 512:
                                first, last = ct == 0, ct == 3
                            else:
                                first, last = ct == 0, ct == NCH - 1
                            nc.tensor.matmul(
                                xh[hh][:, sa:sb_],
                                lhsT=Vh[ct][:, h * E:(h + 1) * E],
                                rhs=ext[:, hh, 0:sb_ - sa],
                                start=first,
                                stop=last,
                            )

                    def take(n):
                        for _ in range(n):
                            if filler:
                                filler.popleft()()

                    for ct in range(NCH):
                        # previous pair's denominator chain: only entries aged
                        # >= 2 ct boundaries, so their dr DMAs have landed and
                        # the reciprocal won't stall the DVE FIFO
                        for ent in deferred:
                            ent[0] -= 1
                        while deferred and deferred[0][0] <= 0:
                            deferred.popleft()[1]()
                        while len(pend) > LAG:
                            pend.popleft()()
                        t0 = ct * P
                        segs = [(512, 1024), (t0, 512)] if t0 < 512 else [(t0, 1024)]
                        entries = []
                        for sa, sb_ in segs:
                            ln = sb_ - sa
                            sc = ps.tile([P, 2, 512], FP32, tag="sc", bufs=1, name=f"sc{c}_{ct}")
                            for hh in range(2):
                                nc.tensor.matmul(
                                    sc[:, hh, :ln],
                                    lhsT=KhT[64 * hh:64 * hh + 64, t0:t0 + P],
                                    rhs=QhT[64 * hh:64 * hh + 64, sa:sb_],
                                    start=True,
                                    stop=True,
                                    tile_position=(64 * hh, 0),
                                )
                            ex = eps.tile([P, 2, 512], BF16, tag="ex", name=f"ex{c}_{ct}")
                            nc.scalar.activation(
                                ex[:, :, :ln], sc[:, :, :ln], AF.Exp, scale=SCALE
                            )
                            if sa == t0:
                                tri_eng.tensor_mul(
                                    ex[:, :, 0:P], ex[:, :, 0:P], tri_sb
                                )
                            entries.append((ex, sa, sb_))
                            take(1)
                            if len(pend) > LAG:
                                pend.popleft()()
                        ex_store[ct] = entries
                        take(1)
                        if len(pend) > LAG:
                            pend.popleft()()
                        for hh in range(2):
                            pend.append(
                                lambda ct=ct, hh=hh, f=emit_attv_half: f(ct, hh)
                            )
                    pend.append(lambda c=c, xh=xh: emit_epilogue(c, xh))
                    take(len(filler))

                # drain: pair 7's attV tail + epilogue + denominator chain,
                # with the rest of pass 1 as PE cover for the chain latency
                while pend:
                    pend.popleft()()
                for st in d1_rest:
                    st()
                while deferred:
                    deferred.popleft()[1]()

            # ---------------- output projection (transposed) ----------------
            # pass 2: add the XT[7] rank-128 update to the bf16 partials
            with ExitStack() as dctx:
                outs = dctx.enter_context(tc.tile_pool(name="outs", bufs=3))
                for n in range(NCH):
                    for half in range(2):
                        op2 = ps.tile([P, 512], FP32, tag="prj", bufs=2, name=f"o2{n}{half}")
                        for kk in (6, 7):
                            nc.tensor.matmul(
                                op2,
                                lhsT=wo_sb[kk][:, n * P:(n + 1) * P],
                                rhs=XT[kk][:, half * 512:(half + 1) * 512],
                                start=(kk == 6),
                                stop=(kk == 7),
                            )
                        ot = outs.tile([P, 512], FP32, tag="ot", name=f"ot{n}{half}")
                        nc.vector.scalar_tensor_tensor(
                            ot, op2, 1.0, dpart[(n, half)],
                            mybir.AluOpType.mult, mybir.AluOpType.add,
                        )
                        nc.sync.dma_start(
                            outd[n * P:(n + 1) * P, half * 512:(half + 1) * 512], ot
                        )

    nc.finalize()
    return nc


def _get_nc():
    key = "v2" + os.environ.get("KMUL_ENG", "gpsimd") + os.environ.get("KTRI_ENG", "gpsimd")
    if key not in _cache:
        _cache[key] = _build(), key
    return _cache[key][0]


def _host_prep(Wq, bq, Wk, bk, Wv, bv, Wo, bo):
    bf = ml_dtypes.bfloat16
    wq_flat = np.ascontiguousarray(
        np.asarray(Wq, np.float32).transpose(1, 0, 2).reshape(DM, DM).astype(bf))
    wk_flat = np.ascontiguousarray(
        np.asarray(Wk, np.float32).transpose(1, 0, 2).reshape(DM, DM).astype(bf))
    wv_flat = np.ascontiguousarray(
        np.asarray(Wv, np.float32).transpose(1, 0, 2).reshape(DM, DM).astype(bf))
    wo_c = np.ascontiguousarray(np.asarray(Wo, np.float32).astype(bf))
    bqf = np.ascontiguousarray(np.asarray(bq, np.float32).reshape(-1))
    bkf = np.ascontiguousarray(np.asarray(bk, np.float32).reshape(-1))
    bop = (
        np.asarray(bo, np.float64)
        + np.asarray(bv, np.float64).reshape(-1) @ np.asarray(Wo, np.float64)
    ).astype(np.float32).reshape(-1)
    return wq_flat, wk_flat, wv_flat, wo_c, bqf, bkf, np.ascontiguousarray(bop)


def _tri2():
    j = np.arange(P)
    keep = (j[None, :] >= j[:, None]).astype(np.float32)  # keep s >= t
    return np.ascontiguousarray(
        np.broadcast_to(keep[:, None, :], (P, 2, P)).astype(ml_dtypes.bfloat16))


def _host_pad0_batch(V, Wv, bv, Wo, bo):
    """Reference semantics for pad==0: uniform attention over ALL keys."""
    V64 = np.asarray(V, np.float64)
    wv_flat = np.asarray(Wv, np.float64).transpose(1, 0, 2).reshape(DM, DM)
    vh = V64 @ wv_flat + np.asarray(bv, np.float64).reshape(-1)  # [S, DM]
    xrow = vh.mean(axis=0)  # [DM]
    orow = xrow @ np.asarray(Wo, np.float64) + np.asarray(bo, np.float64)
    return np.broadcast_to(orow, (S, DM)).astype(np.float32)


def _run(inputs, trace=False):
    Q = np.asarray(inputs["Q"], np.float32)
    K = np.asarray(inputs["K"], np.float32)
    V = np.asarray(inputs["V"], np.float32)
    pad = np.asarray(inputs["pad"]).astype(np.int64)
    wq_flat, wk_flat, wv_flat, wo_c, bqf, bkf, bop = _host_prep(
        inputs["Wq"], inputs["bq"], inputs["Wk"], inputs["bk"],
        inputs["Wv"], inputs["bv"], inputs["Wo"], inputs["bo"],
    )
    tri2 = _tri2()
    bf = ml_dtypes.bfloat16

    nc = _get_nc()

    t = np.arange(S)
    in_maps = []
    for b in range(B):
        qt = np.ascontiguousarray(Q[b].astype(bf).T)
        kt = np.ascontiguousarray(K[b].astype(bf).T)
        vt = np.array(V[b].astype(bf).T)  # writable copy
        p = int(pad[b])
        vones = (t < S - p).astype(np.float32)[:, None] * np.ones((1, H), np.float32)
        if p > 0:
            vt[:, S - p:] = 0
        m = {
            "qt": qt, "kt": kt, "vt": np.ascontiguousarray(vt),
            "wq": wq_flat, "wk": wk_flat, "wv": wv_flat, "wo": wo_c,
            "bqf": bqf, "bkf": bkf, "bop": bop,
            "vones": np.ascontiguousarray(vones.astype(bf)),
            "tri2": tri2,
        }
        in_maps.append(m)

    if trace:
        _install_profile_shim()
    res = run_bass_kernel_spmd(nc, in_maps, list(range(B)), trace=trace)
    out = np.stack([r["out"].T for r in res.results]).astype(np.float32)
    for b in range(B):
        if pad[b] == 0:
            out[b] = _host_pad0_batch(
                V[b], inputs["Wv"], inputs["bv"], inputs["Wo"], inputs["bo"])
    return out, res


def kernel(**inputs):
    out, _ = _run(inputs, trace=bool(os.environ.get("KERNEL_TRACE")))
    return out

